# revision 1
# baseline (speedup 1.0000x reference)
"""Trainium2 Bass kernel: Tacotron-style location-sensitive attention step.

Sharding strategy (8 NeuronCores, SPMD):
  - Attention / conv / softmax / context: pure batch parallelism
    (B=128 -> 16 examples per core; enc_seq, proc_mem, attention weights,
    mask sharded on the batch dim host-side).
  - LSTM cell: H-sharded. Core j computes h.T rows [128j, 128j+128) for the
    FULL batch using only 1/8 of W_ih/W_hh (3.5 MB instead of 28 MB of
    replicated weight traffic), then a small AllGather of h.T.
    qry2 = h @ W_q.T + const is computed for the full batch and each core
    selects its 16 rows with a one-hot matmul (bsel input) so the SPMD
    graph stays core-uniform.

Compute dtypes: bf16 operands for all large matmuls / elementwise (well
inside the 2e-2 rel-err budget), f32 for PSUM, softmax and reductions.

kernel(**inputs) takes FULL numpy inputs (as produced by setup_inputs())
and returns the FULL [128, 512] float32 context.
"""

import sys

sys.path.insert(0, "/opt/trn_rl_repo")

import numpy as np

import concourse.bass as bass
import concourse.mybir as mybir
from concourse import bacc
from concourse.bass_utils import run_bass_kernel_spmd
from concourse.masks import make_identity
from concourse.bass import _add_dep_helper
from concourse.tile import TileContext

F32 = mybir.dt.float32
BF16 = mybir.dt.bfloat16
AF = mybir.ActivationFunctionType

B, S, E, P, H, A, F, KW = 128, 1024, 512, 256, 1024, 128, 32, 31
NCORES = 8
BL = B // NCORES        # 16 examples per core
HL = H // NCORES        # 128 h rows per core
PE_DIM = P + E          # 768
NKI = PE_DIM // 128     # 6
NKH = H // 128          # 8
NC_S = S // 128         # 8 s-chunks
PADW = KW // 2          # 15
CONVROW = PADW + S + 17  # 1056 padded per-channel staging row
TAPS = 2 * KW           # 62
ENC_T = 8               # s-chunks per enc DMA tile (whole example)
ENC_BUFS = 10
GRP = 4                 # examples per fused softmax/context group


def build():
    nc = bacc.Bacc("TRN2", target_bir_lowering=False, debug=False,
                   num_devices=NCORES)

    dp = nc.declare_dram_parameter
    prenet = dp("prenet", [B, P], F32, isOutput=False)
    prev_ctx = dp("prev_ctx", [B, E], F32, isOutput=False)
    att_h = dp("att_h", [B, H], F32, isOutput=False)
    att_c_sh = dp("att_c_sh", [B, HL], F32, isOutput=False)
    w_ih_sh = dp("w_ih_sh", [4, HL, PE_DIM], F32, isOutput=False)
    w_hh_sh = dp("w_hh_sh", [4, HL, H], F32, isOutput=False)
    b_ih_sh = dp("b_ih_sh", [4, HL], F32, isOutput=False)
    b_hh_sh = dp("b_hh_sh", [4, HL], F32, isOutput=False)
    prev_w = dp("prev_w", [BL, S], F32, isOutput=False)
    cum_w = dp("cum_w", [BL, S], F32, isOutput=False)
    enc = dp("enc", [BL, S, E], F32, isOutput=False)
    proc = dp("proc", [BL, S, A], F32, isOutput=False)
    conv_w = dp("conv_w", [F, 2, KW], F32, isOutput=False)
    conv_b = dp("conv_b", [F, 1], F32, isOutput=False)
    w_loc = dp("w_loc", [A, F], F32, isOutput=False)
    b_loc = dp("b_loc", [1, A], F32, isOutput=False)
    w_q = dp("w_q", [A, H], F32, isOutput=False)
    b_q = dp("b_q", [1, A], F32, isOutput=False)
    w_out = dp("w_out", [1, A], F32, isOutput=False)
    bsel = dp("bsel", [B, BL], F32, isOutput=False)
    out = dp("out", [BL, E], F32, isOutput=True)

    with TileContext(nc) as tc:
        with (
            tc.tile_pool(name="const", bufs=1) as cpool,
            tc.tile_pool(name="work", bufs=2) as wpool,
            tc.tile_pool(name="xpadp", bufs=6) as xpool,
            tc.tile_pool(name="conv", bufs=16) as convpool,
            tc.tile_pool(name="proc", bufs=16) as ppool,
            tc.tile_pool(name="vbig", bufs=2) as vpool,
            tc.tile_pool(name="psA", bufs=2, space="PSUM") as psA,
            tc.tile_pool(name="psV", bufs=2, space="PSUM") as psV,
            tc.tile_pool(name="psX", bufs=2, space="PSUM") as psX,
            tc.tile_pool(name="dram", bufs=1, space="DRAM") as dpool,
        ):
            def mm_ps(shape):
                t = psA.tile([128, 512], F32, tag="mm")
                return t[: shape[0], : shape[1]]

            # ------------- constants / small preprocessing -------------
            ident = cpool.tile([128, 128], F32)
            make_identity(nc, ident[:])
            id_bf = cpool.tile([128, 128], BF16)
            nc.vector.tensor_copy(id_bf[:], ident[:])
            ones_row = cpool.tile([1, 128], F32)
            nc.vector.memset(ones_row[:], 1.0)

            pe_t_ctr = [0]

            def pe_t(dst, src_ap, rows, engine=None):
                """dst = src_ap([rows, cols]).T via TensorE (+copy/cast)."""
                ps = mm_ps((dst.shape[0], rows))
                nc.tensor.transpose(ps, src_ap, ident[:rows, :rows])
                if engine is None:
                    pe_t_ctr[0] += 1
                    engine = "dve" if pe_t_ctr[0] % 2 else "act"
                if engine == "dve":
                    nc.vector.tensor_copy(dst, ps)
                else:
                    nc.scalar.copy(dst, ps)

            # ---- LSTM weight shard: load FIRST (DMA priority), PE-T, DVE copy
            # Lives in its own pool, closed after the gates so the SBUF is
            # recycled for the enc prefetch pool.
            NK = NKI + NKH  # 14
            wtpool_cm = tc.tile_pool(name="wt", bufs=1)
            wtpool = wtpool_cm.__enter__()
            wT = wtpool.tile([128, 4, NK, HL], BF16)
            wnats = []
            for g in range(4):
                wi_nat = wtpool.tile([HL, PE_DIM], F32, tag=f"wload{g}")
                nc.sync.dma_start(wi_nat[:], w_ih_sh[g])
                wh_nat = wtpool.tile([HL, H], F32, tag=f"wload2{g}")
                nc.sync.dma_start(wh_nat[:], w_hh_sh[g])
                wnats.append((wi_nat, wh_nat))

            # activations for the LSTM (needed right after W)
            pn_nat = wtpool.tile([B, P], F32)
            nc.sync.dma_start(pn_nat[:], prenet[:])
            pc_nat = wtpool.tile([B, E], F32)
            nc.sync.dma_start(pc_nat[:], prev_ctx[:])
            ah_nat = wtpool.tile([B, H], F32)
            nc.sync.dma_start(ah_nat[:], att_h[:])
            ac_nat = wtpool.tile([B, HL], F32)
            nc.sync.dma_start(ac_nat[:], att_c_sh[:])
            bi_nat = wtpool.tile([4, HL], F32)
            nc.sync.dma_start(bi_nat[:], b_ih_sh[:])
            bh_nat = wtpool.tile([4, HL], F32)
            crit_last = nc.sync.dma_start(bh_nat[:], b_hh_sh[:])

            def pe_t_multi(dst_ap, srcs, rows, engine):
                """Transpose several 128-col chunks into one psA tile, then
                copy them out with a single wide copy."""
                ps = psA.tile([128, 512], F32, tag="mm")
                for i, s_ap in enumerate(srcs):
                    nc.tensor.transpose(ps[:, i * rows:(i + 1) * rows], s_ap,
                                        ident[:rows, :rows])
                if engine == "dve":
                    nc.vector.tensor_copy(dst_ap, ps[:, :len(srcs) * rows])
                else:
                    nc.scalar.copy(dst_ap, ps[:, :len(srcs) * rows])

            for g in range(4):
                wi_nat, wh_nat = wnats[g]
                chunks = [wi_nat[:, k * 128:(k + 1) * 128] for k in range(NKI)]
                chunks += [wh_nat[:, k * 128:(k + 1) * 128] for k in range(NKH)]
                NK2 = NKI + NKH
                for q in range(0, NK2, 4):
                    qs = chunks[q:q + 4]
                    pe_t_multi(wT[:, g, q:q + len(qs), :], qs, HL,
                               "dve" if (q // 4) % 2 else "act")

            inpT = cpool.tile([128, NKI, B], BF16)
            ichunks = [pn_nat[:, k * 128:(k + 1) * 128] for k in range(2)]
            ichunks += [pc_nat[:, k * 128:(k + 1) * 128] for k in range(4)]
            pe_t_multi(inpT[:, 0:4, :], ichunks[0:4], B, "act")
            pe_t_multi(inpT[:, 4:6, :], ichunks[4:6], B, "dve")
            ahT = cpool.tile([128, NKH, B], BF16)
            achunks = [ah_nat[:, k * 128:(k + 1) * 128] for k in range(NKH)]
            pe_t_multi(ahT[:, 0:4, :], achunks[0:4], B, "act")
            pe_t_multi(ahT[:, 4:8, :], achunks[4:8], B, "dve")
            acT = cpool.tile([HL, B], BF16)
            pe_t(acT[:], ac_nat[:], B)
            nc.vector.tensor_add(bi_nat[:], bi_nat[:], bh_nat[:])
            bias_sb = cpool.tile([HL, 4], F32)
            pe_t(bias_sb[:], bi_nat[:], 4)

            # ---- gates (H-shard, full batch) -> h.T shard, as early as possible
            gate_sb = []
            for g in range(4):
                ps = mm_ps((HL, B))
                for k in range(NKI):
                    nc.tensor.matmul(ps, wT[:, g, k, :], inpT[:, k, :],
                                     start=(k == 0), stop=False)
                for k in range(NKH):
                    nc.tensor.matmul(ps, wT[:, g, NKI + k, :], ahT[:, k, :],
                                     start=False, stop=(k == NKH - 1))
                sb = cpool.tile([HL, B], BF16, tag=f"gate{g}")
                fn = AF.Tanh if g == 2 else AF.Sigmoid
                nc.scalar.activation(sb[:], ps, fn, bias=bias_sb[:, g:g + 1])
                gate_sb.append(sb)

            cT = cpool.tile([HL, B], BF16)
            nc.vector.tensor_mul(cT[:], gate_sb[1][:], acT[:])
            tg = cpool.tile([HL, B], BF16)
            nc.vector.tensor_mul(tg[:], gate_sb[0][:], gate_sb[2][:])
            nc.vector.tensor_add(cT[:], cT[:], tg[:])
            nc.scalar.activation(tg[:], cT[:], AF.Tanh)
            hT_sh = cpool.tile([HL, B], BF16)
            nc.vector.tensor_mul(hT_sh[:], gate_sb[3][:], tg[:])
            h_in = dpool.tile([HL, B], BF16)
            nc.scalar.dma_start(h_in[:], hT_sh[:])
            wtpool_cm.__exit__(None, None, None)
            epool_cm = tc.tile_pool(name="enc", bufs=ENC_BUFS)
            epool = epool_cm.__enter__()

            # ---- small constant preprocessing (off the critical path)
            cw_nat = cpool.tile([F, TAPS], F32)
            nc.sync.dma_start(cw_nat[:], conv_w.rearrange("f c k -> f (c k)"))
            w2 = cpool.tile([TAPS, F], BF16)
            pe_t(w2[:], cw_nat[:], F)

            wl_nat = cpool.tile([A, F], F32)
            nc.sync.dma_start(wl_nat[:], w_loc[:])
            wlocT = cpool.tile([F, A], F32)
            pe_t(wlocT[:], wl_nat[:], A)

            cb_col = cpool.tile([F, 1], F32)
            nc.sync.dma_start(cb_col[:], conv_b[:])
            bl_row = cpool.tile([1, A], F32)
            nc.sync.dma_start(bl_row[:], b_loc[:])
            bq_row = cpool.tile([1, A], F32)
            nc.sync.dma_start(bq_row[:], b_q[:])
            ps = mm_ps((1, A))
            nc.tensor.matmul(ps, cb_col[:], wlocT[:], start=True, stop=True)
            const_row = cpool.tile([1, A], F32)
            nc.vector.tensor_add(const_row[:], ps, bl_row[:])
            nc.vector.tensor_add(const_row[:], const_row[:], bq_row[:])

            wo_row = cpool.tile([1, A], F32)
            nc.sync.dma_start(wo_row[:], w_out[:])
            ps = mm_ps((128, A))
            nc.tensor.matmul(ps, ones_row[:], wo_row[:], start=True, stop=True)
            wo_rep8 = cpool.tile([128, NC_S, A], BF16)
            for c in range(NC_S):
                nc.scalar.copy(wo_rep8[:, c, :], ps)

            sel_sb = cpool.tile([B, BL], F32)
            nc.sync.dma_start(sel_sb[:], bsel[:])


            wq_nat = cpool.tile([A, H], F32)
            nc.sync.dma_start(wq_nat[:], w_q[:])
            wqT = cpool.tile([128, NKH, A], BF16)
            qchunks = [wq_nat[:, k * 128:(k + 1) * 128] for k in range(NKH)]
            pe_t_multi(wqT[:, 0:4, :], qchunks[0:4], A, "act")
            pe_t_multi(wqT[:, 4:8, :], qchunks[4:8], A, "dve")

            # padded conv input rows staged to DRAM (bf16):
            # row layout per (b, c): [15 zeros | 1024 data | 17 zeros]
            stage = cpool.tile([BL, 2 * CONVROW], BF16)
            nc.vector.memset(stage[:], 0.0)
            nc.gpsimd.dma_start(stage[:, PADW:PADW + S], cum_w[:])
            nc.gpsimd.dma_start(stage[:, CONVROW + PADW:CONVROW + PADW + S],
                                prev_w[:])
            pad_dram = dpool.tile([BL, 2 * CONVROW], BF16)
            nc.sync.dma_start(pad_dram[:], stage[:])
            # materialize all 62 overlapping window rows per example in DRAM
            win_dram = dpool.tile([BL, TAPS, S], BF16)
            for c in range(2):
                sb2 = pad_dram[0, c * CONVROW:c * CONVROW + 1]
                wsrc = bass.AP(
                    tensor=sb2.tensor,
                    offset=sb2.offset,
                    ap=[[2 * CONVROW, BL], [1, KW], [1, S]],
                )
                db2 = win_dram[0, c * KW:c * KW + 1, 0:1]
                wdst = bass.AP(
                    tensor=db2.tensor,
                    offset=db2.offset,
                    ap=[[TAPS * S, BL], [S, KW], [1, S]],
                )
                nc.sync.dma_start(wdst, wsrc)

            # ---- streaming preloads (bf16 casts on the gpsimd queue)
            proc_tiles = []
            for b in range(6):
                pt = ppool.tile([128, NC_S, A], BF16, tag="proc")
                pdma = nc.gpsimd.dma_start(
                    pt[:], proc[b].rearrange("(p r) a -> p r a", r=NC_S))
                if b == 0:
                    _add_dep_helper(pdma.ins, crit_last.ins, sync=True,
                                    reason="preloads yield DMA BW to LSTM-critical loads")
                proc_tiles.append(pt)
            # ---- location conv (contiguous per-example window loads)
            conv_tiles = []
            for b in range(BL):
                xpadT = xpool.tile([TAPS, S], BF16, tag="xpad")
                nc.sync.dma_start(xpadT[:], win_dram[b])
                conv_sb = convpool.tile([F + 1, S], BF16, tag="conv")
                for h2 in range(2):
                    ps = mm_ps((F, 512))
                    nc.tensor.matmul(ps, w2[:],
                                     xpadT[:, h2 * 512:(h2 + 1) * 512],
                                     start=True, stop=True)
                    nc.scalar.copy(
                        conv_sb[:F, h2 * 512:(h2 + 1) * 512], ps)
                nc.vector.memset(conv_sb[F:F + 1, :], 1.0)
                conv_tiles.append(conv_sb)

            # ---- AllGather h.T (fires as soon as h_in lands)
            h_gat = dpool.tile([NCORES, HL, B], BF16)
            nc.gpsimd.collective_compute(
                "AllGather",
                mybir.AluOpType.bypass,
                replica_groups=[list(range(NCORES))],
                ins=[h_in[:].opt()],
                outs=[h_gat[:].opt()],
            )

            # remaining streams on gpsimd AFTER the collective: their slot
            # stalls resolve through sync/PE/DVE work only (deadlock-safe)
            for b in range(6, BL):
                pt = ppool.tile([128, NC_S, A], BF16, tag="proc")
                nc.gpsimd.dma_start(
                    pt[:], proc[b].rearrange("(p r) a -> p r a", r=NC_S))
                proc_tiles.append(pt)

            enc_tiles = []
            for b in range(10):
                et = epool.tile([128, ENC_T, E], BF16, tag="enc")
                nc.gpsimd.dma_start(
                    et[:], enc[b].rearrange("(p r) e -> p r e", r=NC_S))
                enc_tiles.append(et)


            for b in range(10, BL):
                et = epool.tile([128, ENC_T, E], BF16, tag="enc")
                nc.gpsimd.dma_start(
                    et[:], enc[b].rearrange("(p r) e -> p r e", r=NC_S))
                enc_tiles.append(et)

            hfull = cpool.tile([128, NKH, B], BF16)
            nc.scalar.dma_start(hfull[:], h_gat[:].rearrange("c p b -> p c b"))

            # ---- qry2 (full batch) + batch selection
            ps_q = mm_ps((B, A))
            for k in range(NKH):
                nc.tensor.matmul(ps_q, hfull[:, k, :], wqT[:, k, :],
                                 start=(k == 0), stop=False)
            nc.tensor.matmul(ps_q, ones_row[:], const_row[:],
                             start=False, stop=True)
            qry2_all = cpool.tile([B, A], F32)
            nc.vector.tensor_copy(qry2_all[:], ps_q)
            ps_q2 = mm_ps((BL, A))
            nc.tensor.matmul(ps_q2, sel_sb[:], qry2_all[:],
                             start=True, stop=True)
            qry2 = cpool.tile([BL, A], BF16)
            nc.vector.tensor_copy(qry2[:], ps_q2)

            # rhs_all[:, b, :] = [W_loc.T ; qry2[b]]  (K=33 fused loc+qry mm)
            rhs_all = cpool.tile([F + 1, BL, A], BF16)
            for b in range(BL):
                nc.vector.tensor_copy(rhs_all[:F, b, :], wlocT[:])
            qdram = dpool.tile([BL, A], BF16)
            nc.scalar.dma_start(qdram[:], qry2[:])
            qsrc = bass.AP(
                tensor=qdram[:].tensor,
                offset=qdram[:].offset,
                ap=[[BL * A, 1], [A, BL], [1, A]],
            )
            nc.scalar.dma_start(rhs_all[F:F + 1, :, :], qsrc)

            # ---- fused tail: scores -> group softmax -> context, streaming
            scoresT = cpool.tile([128, NC_S, BL], F32)
            wTt = cpool.tile([128, NC_S, BL], BF16)
            for g in range(BL // GRP):
                bs = range(g * GRP, (g + 1) * GRP)
                for b in bs:
                    conv_sb = conv_tiles[b]
                    ps_v = psV.tile([128, NC_S * A], F32, tag="v")
                    for c in range(NC_S):
                        nc.tensor.matmul(
                            ps_v[:, c * A:(c + 1) * A],
                            conv_sb[:, c:S:NC_S],
                            rhs_all[:, b, :],
                            start=True, stop=True)
                    v_sb = vpool.tile([128, NC_S, A], BF16, tag="v_sb")
                    nc.vector.tensor_add(
                        v_sb[:],
                        ps_v[:].rearrange("p (c a) -> p c a", c=NC_S),
                        proc_tiles[b][:])
                    nc.scalar.activation(v_sb[:], v_sb[:], AF.Tanh)
                    nc.vector.tensor_mul(v_sb[:], v_sb[:], wo_rep8[:])
                    nc.vector.reduce_sum(scoresT[:, :, b], v_sb[:],
                                         axis=mybir.AxisListType.X)

                # group softmax over S in [b, s] layout
                sc = wpool.tile([GRP, S], F32, tag="scg")
                for c in range(NC_S):
                    pe_t(sc[:, c * 128:(c + 1) * 128],
                         scoresT[:, c, g * GRP:(g + 1) * GRP], 128,
                         engine="act")
                mx = wpool.tile([GRP, 1], F32, tag="mxg")
                nc.vector.reduce_max(mx[:], sc[:], axis=mybir.AxisListType.X)
                nc.vector.tensor_scalar_mul(mx[:], mx[:], -1.0)
                sums = wpool.tile([GRP, 1], F32, tag="smg")
                nc.scalar.activation(sc[:], sc[:], AF.Exp, bias=mx[:],
                                     accum_out=sums[:])
                rs = wpool.tile([GRP, 1], F32, tag="rsg")
                nc.vector.reciprocal(rs[:], sums[:])
                nc.vector.tensor_scalar_mul(sc[:], sc[:], rs[:])
                for c in range(NC_S):
                    pe_t(wTt[:, c, g * GRP:(g + 1) * GRP],
                         sc[:, c * 128:(c + 1) * 128], GRP, engine="act")

                # context for this group
                for b in bs:
                    ps_x = psX.tile([1, E], F32, tag="ctx")
                    for c in range(NC_S):
                        nc.tensor.matmul(ps_x, wTt[:, c, b:b + 1],
                                         enc_tiles[b][:, c, :],
                                         start=(c == 0), stop=(c == NC_S - 1))
                    ctx_row = wpool.tile([1, E], F32, tag="ctxrow")
                    nc.scalar.copy(ctx_row[:], ps_x)
                    nc.sync.dma_start(out[b:b + 1, :], ctx_row[:])

            epool_cm.__exit__(None, None, None)

    nc.compile()
    return nc


_NC_CACHE = None


def _get_nc():
    global _NC_CACHE
    if _NC_CACHE is None:
        _NC_CACHE = build()
    return _NC_CACHE


def shard_inputs(prenet, prev_context, att_h, att_c, prev_weights, cum_weights,
                 enc_seq, proc_mem, mask, W_ih, W_hh, b_ih, b_hh, conv_w,
                 conv_b, W_loc, b_loc, W_q, b_q, W_out, **_unused):
    f = np.ascontiguousarray
    w_ih4 = np.asarray(W_ih, np.float32).reshape(4, H, PE_DIM)
    w_hh4 = np.asarray(W_hh, np.float32).reshape(4, H, H)
    b_ih4 = np.asarray(b_ih, np.float32).reshape(4, H)
    b_hh4 = np.asarray(b_hh, np.float32).reshape(4, H)
    in_maps = []
    for j in range(NCORES):
        bj = slice(BL * j, BL * (j + 1))
        hj = slice(HL * j, HL * (j + 1))
        sel = np.zeros((B, BL), np.float32)
        sel[BL * j:BL * (j + 1), :] = np.eye(BL, dtype=np.float32)
        in_maps.append({
            "prenet": f(np.asarray(prenet, np.float32)),
            "prev_ctx": f(np.asarray(prev_context, np.float32)),
            "att_h": f(np.asarray(att_h, np.float32)),
            "att_c_sh": f(np.asarray(att_c, np.float32)[:, hj]),
            "w_ih_sh": f(w_ih4[:, hj]),
            "w_hh_sh": f(w_hh4[:, hj]),
            "b_ih_sh": f(b_ih4[:, hj]),
            "b_hh_sh": f(b_hh4[:, hj]),
            "prev_w": f(np.asarray(prev_weights, np.float32)[bj]),
            "cum_w": f(np.asarray(cum_weights, np.float32)[bj]),
            "enc": f(np.asarray(enc_seq, np.float32)[bj]),
            "proc": f(np.asarray(proc_mem, np.float32)[bj]),
            "conv_w": f(np.asarray(conv_w, np.float32)),
            "conv_b": f(np.asarray(conv_b, np.float32).reshape(F, 1)),
            "w_loc": f(np.asarray(W_loc, np.float32)),
            "b_loc": f(np.asarray(b_loc, np.float32).reshape(1, A)),
            "w_q": f(np.asarray(W_q, np.float32)),
            "b_q": f(np.asarray(b_q, np.float32).reshape(1, A)),
            "w_out": f(np.asarray(W_out, np.float32).reshape(1, A)),
            "bsel": sel,
        })
    return in_maps


def kernel(**inputs):
    assert not np.any(np.asarray(inputs["mask"])), \
        "kernel assumes mask == 0 (softmax-shift support not implemented)"
    nc = _get_nc()
    in_maps = shard_inputs(**inputs)
    res = run_bass_kernel_spmd(nc, in_maps, core_ids=list(range(NCORES)))
    return np.concatenate([res.results[j]["out"] for j in range(NCORES)],
                          axis=0)


if __name__ == "__main__":
    rng = np.random.default_rng(0)
    print("building...")
    _get_nc()
    print("built ok")



# revision 3
# speedup vs baseline: 1.2000x; 1.2000x over previous
"""Trainium2 Bass kernel: Tacotron-style location-sensitive attention step.

Sharding (8 NeuronCores, SPMD):
  - Attention / conv / softmax / context: batch parallel (B=128 -> 16
    examples per core; enc_seq, proc_mem, conv inputs batch-sharded).
  - LSTM cell: H-sharded. Core j computes h.T rows [128j, 128j+128) for
    the FULL batch with 1/8 of W_ih/W_hh, then computes a PARTIAL
    qry2 = h_shard @ W_q_shard.T for the full batch and a ReduceScatter
    sums the partials while scattering the batch dim, so each core ends
    with the final qry2 rows for its own 16 examples.

Layout choices:
  - Location features are computed transposed: loc.T[a, s] =
    (W_loc @ conv)[a, s] by folding W_loc into the conv weights
    (WfoldT[62, A]), with the conv input as 62 overlapping window rows
    DMA'd straight from a host-padded [BL, 2, S+30] tensor.
  - proc_mem is host-transposed to [BL, A, S] so v = qry + proc + loc
    lives in [A=128 partitions, S] layout; the qry2 add becomes a
    per-partition ACT bias fused into the tanh.
  - scores = w_out . v via two N=512 matmuls per example; groups of 4
    examples land in one PSUM bank at partitions {0,32,64,96} via
    matmul tile_position, giving partition-parallel softmax.
  - context = weights @ enc with bf16 enc tiles (full residency, the
    DMA stream starts at t=0 and never stalls on consumption).

kernel(**inputs) takes FULL numpy inputs and returns [128, 512] f32.
"""

import sys

sys.path.insert(0, "/opt/trn_rl_repo")

import numpy as np

import concourse.bass as bass
import concourse.mybir as mybir
from concourse import bacc
from concourse.bass_utils import run_bass_kernel_spmd
from concourse.masks import make_identity
from concourse.tile import TileContext

F32 = mybir.dt.float32
BF16 = mybir.dt.bfloat16
AF = mybir.ActivationFunctionType

B, S, E, P, H, A, F, KW = 128, 1024, 512, 256, 1024, 128, 32, 31
NCORES = 8
BL = B // NCORES        # 16 examples per core
HL = H // NCORES        # 128 h rows per core
PE_DIM = P + E          # 768
NKI = PE_DIM // 128     # 6
NKH = H // 128          # 8
NC_S = S // 128         # 8 s-chunks
PADW = KW // 2          # 15
PADL = S + 2 * PADW     # 1054 padded conv row
TAPS = 2 * KW           # 62
ENC_BUFS = 12
XH = BL // 2            # 8 examples per xpad half


def build():
    nc = bacc.Bacc("TRN2", target_bir_lowering=False, debug=False,
                   num_devices=NCORES)

    dp = nc.declare_dram_parameter
    prenet = dp("prenet", [B, P], F32, isOutput=False)
    prev_ctx = dp("prev_ctx", [B, E], F32, isOutput=False)
    att_h = dp("att_h", [B, H], F32, isOutput=False)
    att_c_sh = dp("att_c_sh", [B, HL], F32, isOutput=False)
    w_ih_sh = dp("w_ih_sh", [4, HL, PE_DIM], F32, isOutput=False)
    w_hh_sh = dp("w_hh_sh", [4, HL, H], F32, isOutput=False)
    b_sh = dp("b_sh", [4, HL], F32, isOutput=False)
    loc_pad = dp("loc_pad", [BL, 2, PADL], F32, isOutput=False)
    enc = dp("enc", [BL, S, E], F32, isOutput=False)
    procT = dp("procT", [BL, A, S], F32, isOutput=False)
    conv_w = dp("conv_w", [F, 2, KW], F32, isOutput=False)
    cb_col = dp("cb_col", [F, 1], F32, isOutput=False)
    w_loc = dp("w_loc", [A, F], F32, isOutput=False)
    blq_col = dp("blq_col", [A, 1], F32, isOutput=False)
    wq_shT = dp("wq_shT", [HL, A], F32, isOutput=False)
    wo_colT = dp("wo_colT", [A, 1], F32, isOutput=False)
    out = dp("out", [BL, E], F32, isOutput=True)

    with TileContext(nc) as tc:
        with (
            tc.tile_pool(name="const", bufs=1) as cpool,
            tc.tile_pool(name="work", bufs=2) as wpool,
            tc.tile_pool(name="xpadp", bufs=1) as xpool,
            tc.tile_pool(name="proc", bufs=16) as ppool,
            tc.tile_pool(name="enc", bufs=ENC_BUFS) as epool,
            tc.tile_pool(name="psA", bufs=2, space="PSUM") as psA,
            tc.tile_pool(name="psL", bufs=1, space="PSUM") as psL,
            tc.tile_pool(name="psS", bufs=2, space="PSUM") as psS,
            tc.tile_pool(name="psX", bufs=2, space="PSUM") as psX,
            tc.tile_pool(name="dram", bufs=1, space="DRAM") as dpool,
        ):
            def mm_ps(shape):
                t = psA.tile([128, 512], F32, tag="mm")
                return t[: shape[0], : shape[1]]

            # ------------- constants -------------
            ident = cpool.tile([128, 128], F32)
            make_identity(nc, ident[:])

            pe_t_ctr = [0]

            def pe_t(dst, src_ap, rows, engine=None):
                """dst = src_ap([rows, cols]).T via TensorE (+copy/cast)."""
                ps = mm_ps((dst.shape[0], rows))
                nc.tensor.transpose(ps, src_ap, ident[:rows, :rows])
                if engine is None:
                    pe_t_ctr[0] += 1
                    engine = "dve" if pe_t_ctr[0] % 2 else "act"
                if engine == "dve":
                    nc.vector.tensor_copy(dst, ps)
                else:
                    nc.scalar.copy(dst, ps)

            def pe_t_multi(dst_ap, srcs, rows, engine):
                ps = psA.tile([128, 512], F32, tag="mm")
                for i, s_ap in enumerate(srcs):
                    nc.tensor.transpose(ps[:, i * rows:(i + 1) * rows], s_ap,
                                        ident[:rows, :rows])
                if engine == "dve":
                    nc.vector.tensor_copy(dst_ap, ps[:, :len(srcs) * rows])
                else:
                    nc.scalar.copy(dst_ap, ps[:, :len(srcs) * rows])

            # ---- LSTM weight shard staging (double-buffered pairs) + acts
            NK = NKI + NKH  # 14
            wtpool_cm = tc.tile_pool(name="wt", bufs=1)
            wtpool = wtpool_cm.__enter__()
            wT = wtpool.tile([128, 4, NK, HL], BF16)

            def load_w(g):
                wi_nat = wtpool.tile([HL, PE_DIM], F32, tag=f"wi{g % 2}")
                nc.sync.dma_start(wi_nat[:], w_ih_sh[g])
                wh_nat = wtpool.tile([HL, H], F32, tag=f"wh{g % 2}")
                nc.sync.dma_start(wh_nat[:], w_hh_sh[g])
                return wi_nat, wh_nat

            wnats = [load_w(0), load_w(1)]

            pn_nat = wtpool.tile([B, P], F32)
            nc.sync.dma_start(pn_nat[:], prenet[:])
            pc_nat = wtpool.tile([B, E], F32)
            nc.sync.dma_start(pc_nat[:], prev_ctx[:])
            ah_nat = wtpool.tile([B, H], F32)
            nc.sync.dma_start(ah_nat[:], att_h[:])
            ac_nat = wtpool.tile([B, HL], F32)
            nc.sync.dma_start(ac_nat[:], att_c_sh[:])
            bs_nat = wtpool.tile([4, HL], F32)
            nc.sync.dma_start(bs_nat[:], b_sh[:])

            # ---- gpsimd (swdge) stream, block A: conv windows, procT, enc
            def xpad_dma(hb):
                xp = xpool.tile([TAPS, XH, S], BF16, tag="xp")
                for c in range(2):
                    sl = loc_pad[XH * hb, c, 0:1]
                    src = bass.AP(
                        tensor=sl.tensor,
                        offset=sl.offset,
                        ap=[[1, KW], [2 * PADL, XH], [1, S]],
                    )
                    nc.gpsimd.dma_start(xp[c * KW:(c + 1) * KW], src)
                return xp

            xp1 = xpad_dma(0)

            proc_tiles = []
            for b in range(8):
                pt = ppool.tile([A, S], BF16, tag="proc")
                nc.gpsimd.dma_start(pt[:], procT[b])
                proc_tiles.append(pt)

            enc_tiles = []
            for b in range(2):
                et = epool.tile([128, NC_S, E], BF16, tag="enc")
                nc.gpsimd.dma_start(
                    et[:], enc[b].rearrange("(p r) e -> p r e", r=NC_S))
                enc_tiles.append(et)

            # ---- small consts on the scalar (ACT hwdge) queue
            cw_nat = cpool.tile([F, TAPS], F32)
            nc.scalar.dma_start(cw_nat[:], conv_w.rearrange("f c k -> f (c k)"))
            wl_nat = cpool.tile([A, F], F32)
            nc.scalar.dma_start(wl_nat[:], w_loc[:])
            cb_sb = cpool.tile([F, 1], F32)
            nc.scalar.dma_start(cb_sb[:], cb_col[:])
            blq_sb = cpool.tile([A, 1], F32)
            nc.scalar.dma_start(blq_sb[:], blq_col[:])
            wq_nat = cpool.tile([HL, A], F32)
            nc.scalar.dma_start(wq_nat[:], wq_shT[:])
            wo_nat = cpool.tile([A, 1], F32)
            nc.scalar.dma_start(wo_nat[:], wo_colT[:])

            # ---- LSTM front: W transposes (g0/g1), input transposes,
            #      then W g2/g3 behind them, gates, state update.
            def tr_w(g, wi_nat, wh_nat):
                chunks = [wi_nat[:, k * 128:(k + 1) * 128] for k in range(NKI)]
                chunks += [wh_nat[:, k * 128:(k + 1) * 128]
                           for k in range(NKH)]
                for q in range(0, NK, 4):
                    qs = chunks[q:q + 4]
                    pe_t_multi(wT[:, g, q:q + len(qs), :], qs, HL,
                               "dve" if (q // 4) % 2 else "act")

            tr_w(0, *wnats[0])
            tr_w(1, *wnats[1])

            inpT = wtpool.tile([128, NKI, B], BF16)
            ichunks = [pn_nat[:, k * 128:(k + 1) * 128] for k in range(2)]
            ichunks += [pc_nat[:, k * 128:(k + 1) * 128] for k in range(4)]
            pe_t_multi(inpT[:, 0:4, :], ichunks[0:4], B, "act")
            pe_t_multi(inpT[:, 4:6, :], ichunks[4:6], B, "dve")
            ahT = wtpool.tile([128, NKH, B], BF16)
            achunks = [ah_nat[:, k * 128:(k + 1) * 128] for k in range(NKH)]
            pe_t_multi(ahT[:, 0:4, :], achunks[0:4], B, "act")
            pe_t_multi(ahT[:, 4:8, :], achunks[4:8], B, "dve")
            acT = wtpool.tile([HL, B], BF16)
            pe_t(acT[:], ac_nat[:], B)
            bias_sb = wtpool.tile([HL, 4], F32)
            pe_t(bias_sb[:], bs_nat[:], 4)

            wnats += [load_w(2), load_w(3)]

            gate_sb = []
            for g in range(2):
                ps = mm_ps((HL, B))
                for k in range(NKI):
                    nc.tensor.matmul(ps, wT[:, g, k, :], inpT[:, k, :],
                                     start=(k == 0), stop=False)
                for k in range(NKH):
                    nc.tensor.matmul(ps, wT[:, g, NKI + k, :], ahT[:, k, :],
                                     start=False, stop=(k == NKH - 1))
                sb = wtpool.tile([HL, B], BF16, tag=f"gate{g}")
                fn = AF.Tanh if g == 2 else AF.Sigmoid
                nc.scalar.activation(sb[:], ps, fn, bias=bias_sb[:, g:g + 1])
                gate_sb.append(sb)

            tr_w(2, *wnats[2])
            tr_w(3, *wnats[3])
            for g in range(2, 4):
                ps = mm_ps((HL, B))
                for k in range(NKI):
                    nc.tensor.matmul(ps, wT[:, g, k, :], inpT[:, k, :],
                                     start=(k == 0), stop=False)
                for k in range(NKH):
                    nc.tensor.matmul(ps, wT[:, g, NKI + k, :], ahT[:, k, :],
                                     start=False, stop=(k == NKH - 1))
                sb = wtpool.tile([HL, B], BF16, tag=f"gate{g}")
                fn = AF.Tanh if g == 2 else AF.Sigmoid
                nc.scalar.activation(sb[:], ps, fn, bias=bias_sb[:, g:g + 1])
                gate_sb.append(sb)

            cT = wtpool.tile([HL, B], BF16)
            nc.vector.tensor_mul(cT[:], gate_sb[1][:], acT[:])
            tg = wtpool.tile([HL, B], BF16)
            nc.vector.tensor_mul(tg[:], gate_sb[0][:], gate_sb[2][:])
            nc.vector.tensor_add(cT[:], cT[:], tg[:])
            nc.scalar.activation(tg[:], cT[:], AF.Tanh)
            hT_sh = wtpool.tile([HL, B], BF16)
            nc.vector.tensor_mul(hT_sh[:], gate_sb[3][:], tg[:])

            # ---- partial qry2 for the FULL batch from this h shard
            wq_bf = wtpool.tile([HL, A], BF16)
            nc.vector.tensor_copy(wq_bf[:], wq_nat[:])
            ps_q = mm_ps((B, A))
            nc.tensor.matmul(ps_q, hT_sh[:], wq_bf[:], start=True, stop=True)
            qp_sb = wtpool.tile([B, A], F32)
            nc.vector.tensor_copy(qp_sb[:], ps_q)
            qp_dram = dpool.tile([B, A], F32)
            nc.sync.dma_start(qp_dram[:], qp_sb[:])
            wtpool_cm.__exit__(None, None, None)

            # ---- ReduceScatter: sum qry2 partials over cores, scatter batch
            qrs_dram = dpool.tile([BL, A], F32)
            nc.gpsimd.collective_compute(
                "ReduceScatter",
                mybir.AluOpType.add,
                replica_groups=[list(range(NCORES))],
                ins=[qp_dram[:].opt()],
                outs=[qrs_dram[:].opt()],
            )

            # ---- gpsimd stream, block B (generation resumes after RS fires)
            xp2 = xpad_dma(1)
            for b in range(8, BL):
                pt = ppool.tile([A, S], BF16, tag="proc")
                nc.gpsimd.dma_start(pt[:], procT[b])
                proc_tiles.append(pt)
            for b in range(2, BL):
                et = epool.tile([128, NC_S, E], BF16, tag="enc")
                nc.gpsimd.dma_start(
                    et[:], enc[b].rearrange("(p r) e -> p r e", r=NC_S))
                enc_tiles.append(et)

            # ---- conv/location precompute: WfoldT = conv_w.T @ W_loc.T
            wlocT = cpool.tile([F, A], F32)
            pe_t(wlocT[:], wl_nat[:], A, engine="dve")
            ps_f = mm_ps((TAPS, A))
            nc.tensor.matmul(ps_f, cw_nat[:], wlocT[:], start=True, stop=True)
            wfold = cpool.tile([TAPS, A], BF16)
            nc.scalar.copy(wfold[:], ps_f)
            # constT = (W_loc @ conv_b + b_loc + b_q).T  [A, 1]
            ps_c = mm_ps((A, 1))
            nc.tensor.matmul(ps_c, wlocT[:], cb_sb[:], start=True, stop=True)
            constT = cpool.tile([A, 1], F32)
            nc.vector.tensor_add(constT[:], ps_c, blq_sb[:])
            wo_bf = cpool.tile([A, 1], BF16)
            nc.vector.tensor_copy(wo_bf[:], wo_nat[:])

            # ---- v_pre[b] = loc.T + proc.T  (in [A, S] layout, bf16)
            for b in range(BL):
                xp = xp1 if b < XH else xp2
                bb = b % XH
                ps = psL.tile([128, S], F32, tag="loc")
                nc.tensor.matmul(ps[:, 0:512], wfold[:], xp[:, bb, 0:512],
                                 start=True, stop=True)
                nc.tensor.matmul(ps[:, 512:1024], wfold[:], xp[:, bb, 512:1024],
                                 start=True, stop=True)
                nc.vector.tensor_add(proc_tiles[b][:], ps[:], proc_tiles[b][:])

            # ---- final qry2 rows for this core's 16 examples
            qrs_sb = cpool.tile([BL, A], F32)
            nc.scalar.dma_start(qrs_sb[:], qrs_dram[:])
            qry2T = cpool.tile([A, BL], F32)
            pe_t(qry2T[:], qrs_sb[:], BL, engine="act")
            nc.vector.tensor_scalar_add(qry2T[:], qry2T[:], constT[:])

            # ---- tail: tanh -> scores -> group softmax -> context
            for g in range(BL // 4):
                bs = list(range(g * 4, (g + 1) * 4))
                ps_a = psS.tile([128, 512], F32, tag="sc")
                ps_b = psS.tile([128, 512], F32, tag="sc")
                for i, b in enumerate(bs):
                    pt = proc_tiles[b]
                    nc.scalar.activation(pt[:], pt[:], AF.Tanh,
                                         bias=qry2T[:, b:b + 1])
                    row = 32 * i
                    nc.tensor.matmul(ps_a[row:row + 1, :], wo_bf[:],
                                     pt[:, 0:512], start=True, stop=True,
                                     tile_position=(0, row))
                    nc.tensor.matmul(ps_b[row:row + 1, :], wo_bf[:],
                                     pt[:, 512:1024], start=True, stop=True,
                                     tile_position=(0, row))

                # softmax over s; rows {0,32,64,96} hold the 4 examples,
                # other partitions carry garbage that is never read.
                sc_g = wpool.tile([128, S], F32, tag="scg")
                nc.vector.tensor_copy(sc_g[:, 0:512], ps_a[:])
                nc.scalar.copy(sc_g[:, 512:1024], ps_b[:])
                mx = wpool.tile([128, 1], F32, tag="mxg")
                nc.vector.reduce_max(mx[:], sc_g[:], axis=mybir.AxisListType.X)
                nc.vector.tensor_scalar_mul(mx[:], mx[:], -1.0)
                sums = wpool.tile([128, 1], F32, tag="smg")
                nc.scalar.activation(sc_g[:], sc_g[:], AF.Exp, bias=mx[:],
                                     accum_out=sums[:])
                rs = wpool.tile([128, 1], F32, tag="rsg")
                nc.vector.reciprocal(rs[:], sums[:])
                nc.vector.tensor_scalar_mul(sc_g[:], sc_g[:], rs[:])

                # transpose weights back: wTt[:, c, i] = weight(s=8p+c, ex i)
                wTt = wpool.tile([128, NC_S, 4], BF16, tag="wtt")
                for h2 in range(2):
                    ps = psA.tile([128, 512], F32, tag="mm")
                    for cc in range(4):
                        c = 4 * h2 + cc
                        nc.tensor.transpose(ps[:, cc * 128:(cc + 1) * 128],
                                            sc_g[:, c:S:NC_S],
                                            ident[:, :])
                    ps_r = ps[:].rearrange("p (c x) -> p c x", c=4)
                    if h2 == 0:
                        nc.vector.tensor_copy(wTt[:, 0:4, :],
                                              ps_r[:, :, 0:128:32])
                    else:
                        nc.scalar.copy(wTt[:, 4:8, :], ps_r[:, :, 0:128:32])

                for i, b in enumerate(bs):
                    ps_x = psX.tile([1, E], F32, tag="ctx")
                    for c in range(NC_S):
                        nc.tensor.matmul(ps_x, wTt[:, c, i:i + 1],
                                         enc_tiles[b][:, c, :],
                                         start=(c == 0), stop=(c == NC_S - 1))
                    ctx_row = wpool.tile([1, E], F32, tag="ctxrow")
                    if i % 2 == 0:
                        nc.vector.tensor_copy(ctx_row[:], ps_x)
                    else:
                        nc.scalar.copy(ctx_row[:], ps_x)
                    nc.sync.dma_start(out[b:b + 1, :], ctx_row[:])

    nc.compile()
    return nc


_NC_CACHE = None


def _get_nc():
    global _NC_CACHE
    if _NC_CACHE is None:
        _NC_CACHE = build()
    return _NC_CACHE


def shard_inputs(prenet, prev_context, att_h, att_c, prev_weights, cum_weights,
                 enc_seq, proc_mem, mask, W_ih, W_hh, b_ih, b_hh, conv_w,
                 conv_b, W_loc, b_loc, W_q, b_q, W_out, **_unused):
    f = np.ascontiguousarray
    w_ih4 = np.asarray(W_ih, np.float32).reshape(4, H, PE_DIM)
    w_hh4 = np.asarray(W_hh, np.float32).reshape(4, H, H)
    b4 = (np.asarray(b_ih, np.float32)
          + np.asarray(b_hh, np.float32)).reshape(4, H)
    blq = (np.asarray(b_loc, np.float32).reshape(A)
           + np.asarray(b_q, np.float32).reshape(A)).reshape(A, 1)
    in_maps = []
    for j in range(NCORES):
        bj = slice(BL * j, BL * (j + 1))
        hj = slice(HL * j, HL * (j + 1))
        lp = np.zeros((BL, 2, PADL), np.float32)
        lp[:, 0, PADW:PADW + S] = np.asarray(cum_weights, np.float32)[bj]
        lp[:, 1, PADW:PADW + S] = np.asarray(prev_weights, np.float32)[bj]
        in_maps.append({
            "prenet": f(np.asarray(prenet, np.float32)),
            "prev_ctx": f(np.asarray(prev_context, np.float32)),
            "att_h": f(np.asarray(att_h, np.float32)),
            "att_c_sh": f(np.asarray(att_c, np.float32)[:, hj]),
            "w_ih_sh": f(w_ih4[:, hj]),
            "w_hh_sh": f(w_hh4[:, hj]),
            "b_sh": f(b4[:, hj]),
            "loc_pad": lp,
            "enc": f(np.asarray(enc_seq, np.float32)[bj]),
            "procT": f(np.asarray(proc_mem, np.float32)[bj].transpose(0, 2, 1)),
            "conv_w": f(np.asarray(conv_w, np.float32)),
            "cb_col": f(np.asarray(conv_b, np.float32).reshape(F, 1)),
            "w_loc": f(np.asarray(W_loc, np.float32)),
            "blq_col": blq,
            "wq_shT": f(np.asarray(W_q, np.float32)[:, hj].T),
            "wo_colT": f(np.asarray(W_out, np.float32).reshape(1, A).T),
            "out": np.zeros((BL, E), np.float32),
        })
    return in_maps


def kernel(**inputs):
    assert not np.any(np.asarray(inputs["mask"])), \
        "kernel assumes mask == 0 (softmax-shift support not implemented)"
    nc = _get_nc()
    in_maps = shard_inputs(**inputs)
    for m in in_maps:
        m.pop("out", None)
    res = run_bass_kernel_spmd(nc, in_maps, core_ids=list(range(NCORES)))
    return np.concatenate([res.results[j]["out"] for j in range(NCORES)],
                          axis=0)


if __name__ == "__main__":
    print("building...")
    _get_nc()
    print("built ok")
